# revision 27
# baseline (speedup 1.0000x reference)
"""v6: mixed-precision E-fused attention. Six symmetric attentions in fp8
(e4m3) DoubleRow; the sharp-softmax cross attention on bf16. Transposed-score
layout (exp emits probsT directly), softmax sums via interleaved N=1 matmuls,
PV normalized in the PSUM->SBUF copy, residual combines deferred two pipeline
stages with bf16 adds (DVE 2x), bf16 outputs upcast on host. PV of the
previous unit is interleaved into the scores loop to hide exp latency.
Data-parallel: 2 batches/core, 8 cores."""

import numpy as np
import ml_dtypes

from concourse import bacc, bass, tile, mybir
from concourse.bass_utils import run_bass_kernel_spmd

B, L, D = 16, 1024, 512
A = D
NCORES = 8
BLOC = B // NCORES
P = 128
DC = D // P          # 4 contraction chunks of 128
LT = L // P          # 8 row blocks
KC = L // P          # 8 key blocks
SCALE = float(1.0 / np.sqrt(np.float32(D)))
WSC = 16.0           # host weight pre-scale for fp8 (subnormal avoidance)
EB = -2.0            # exp bias: probs scaled by e^-2, cancels in softmax

F32 = mybir.dt.float32
BF16 = mybir.dt.bfloat16
FP8 = mybir.dt.float8e4
DR = mybir.MatmulPerfMode.DoubleRow
EXP = mybir.ActivationFunctionType.Exp
COPY = mybir.ActivationFunctionType.Copy


def _build():
    nc = bacc.Bacc("TRN2", target_bir_lowering=False, debug=False,
                   num_devices=NCORES)

    mt_txt = nc.dram_tensor("mt_txt", (BLOC, D, L), FP8, kind="ExternalInput").ap()
    mt_au = nc.dram_tensor("mt_au", (BLOC, D, L), FP8, kind="ExternalInput").ap()
    mt_vi = nc.dram_tensor("mt_vi", (BLOC, D, L), FP8, kind="ExternalInput").ap()
    mt_txtb = nc.dram_tensor("mt_txtb", (BLOC, D, L), BF16,
                             kind="ExternalInput").ap()
    res = nc.dram_tensor("res", (3, BLOC, L, D), BF16, kind="ExternalInput").ap()
    wt = nc.dram_tensor("wt", (12, D, A), FP8, kind="ExternalInput").ap()
    wtb = nc.dram_tensor("wtb", (2, D, A), BF16, kind="ExternalInput").ap()
    out = nc.dram_tensor("out", (BLOC, L, 4 * A), BF16, kind="ExternalOutput").ap()

    with tile.TileContext(nc) as tc:
        _body(nc, tc, mt_txt, mt_au, mt_vi, mt_txtb, res, wt, wtb, out)

    nc.compile()
    return nc


def _body(nc, tc, mt_txt, mt_au, mt_vi, mt_txtb, res, wt, wtb, out):
    mt_dram = {"txt": mt_txt, "au": mt_au, "vi": mt_vi}

    with (
        tc.tile_pool(name="persist", bufs=1) as persist,
        tc.tile_pool(name="wpool", bufs=2) as wpool,
        tc.tile_pool(name="work", bufs=2) as work,
        tc.tile_pool(name="small", bufs=3) as smallp,
        tc.tile_pool(name="ps_score", bufs=2, space=bass.MemorySpace.PSUM) as psA,
        tc.tile_pool(name="ps_mm", bufs=3, space=bass.MemorySpace.PSUM) as psB,
        tc.tile_pool(name="ps_sums", bufs=1, space=bass.MemorySpace.PSUM) as psC,
    ):
        # --- persistent tiles; first-needed inputs first, weights on the
        # scalar DMA queue so they issue in parallel with the sync queue ------
        mtT = {}

        def load_mt(name, b):
            t = persist.tile([P, DC, L], FP8, tag=f"mt_{name}{b}",
                             name=f"mt_{name}{b}")
            nc.sync.dma_start(
                out=t[:, :, :],
                in_=mt_dram[name][b].rearrange("(dc p) l -> p dc l", p=P))
            mtT[(name, b)] = t

        wtiles = {}

        def use_w(j, tag):
            # j >= 12 -> bf16 cross weights from wtb
            if j not in wtiles:
                if j >= 12:
                    t = wpool.tile([P, DC, A], BF16, tag=tag, name=f"w{j}")
                    nc.scalar.dma_start(
                        out=t[:, :, :],
                        in_=wtb[j - 12].rearrange("(dc p) a -> p dc a", p=P))
                else:
                    t = wpool.tile([P, DC, A], FP8, tag=tag, name=f"w{j}")
                    nc.scalar.dma_start(
                        out=t[:, :, :],
                        in_=wt[j].rearrange("(dc p) a -> p dc a", p=P))
                wtiles[j] = t
            return wtiles[j]

        for j, tag in ((0, "E1"), (2, "Wv1"), (1, "E2"), (3, "Wv2")):
            use_w(j, tag)
        load_mt("txt", 0)
        load_mt("au", 0)
        load_mt("vi", 0)
        load_mt("txt", 1)
        load_mt("au", 1)
        load_mt("vi", 1)
        txtb = []
        for b in range(BLOC):
            t = persist.tile([P, DC, L], BF16, tag=f"txtb{b}", name=f"txtb{b}")
            nc.sync.dma_start(
                out=t[:, :, :],
                in_=mt_txtb[b].rearrange("(dc p) l -> p dc l", p=P))
            txtb.append(t)

        avT = [persist.tile([P, DC, L], BF16, tag=f"avT{b}", name=f"avT{b}")
               for b in range(BLOC)]
        onesDR = persist.tile([P, 2, 1], FP8, tag="onesDR", name="onesDR")
        nc.vector.memset(onesDR[:, :, :], WSC)
        ones_bf = persist.tile([P, 1], BF16, tag="ones_bf", name="ones_bf")
        nc.vector.memset(ones_bf[:, :], 1.0)
        ebias = persist.tile([P, 1], F32, tag="ebias", name="ebias")
        nc.vector.memset(ebias[:, :], EB)

        # --- per-unit phases --------------------------------------------------
        def proj_T(E, mqT, fp8):
            tT = work.tile([P, DC, L], FP8 if fp8 else BF16, tag="tT", name="tT")
            for db in range(DC):
                for qh in range(2):
                    ps = psB.tile([P, 512], F32, tag="mm", name="ps_pt")
                    if fp8:
                        for dcp in (0, 2):
                            nc.tensor.matmul(
                                ps[:, :],
                                E[:, dcp:dcp + 2, db * P:(db + 1) * P],
                                mqT[:, dcp:dcp + 2, qh * 512:(qh + 1) * 512],
                                start=(dcp == 0), stop=(dcp == 2), perf_mode=DR)
                    else:
                        for dc in range(DC):
                            nc.tensor.matmul(
                                ps[:, :],
                                E[:, dc, db * P:(db + 1) * P],
                                mqT[:, dc, qh * 512:(qh + 1) * 512],
                                start=(dc == 0), stop=(dc == DC - 1))
                    nc.vector.tensor_copy(tT[:, db, qh * 512:(qh + 1) * 512],
                                          ps[:, :])
            return tT

        def proj_N(WvT, mkvT, fp8):
            v = work.tile([P, KC, A], FP8 if fp8 else BF16, tag="v", name="v")
            for lt in range(LT):
                ps = psB.tile([P, A], F32, tag="mm", name="ps_pn")
                if fp8:
                    for dcp in (0, 2):
                        nc.tensor.matmul(ps[:, :],
                                         mkvT[:, dcp:dcp + 2, lt * P:(lt + 1) * P],
                                         WvT[:, dcp:dcp + 2, :],
                                         start=(dcp == 0), stop=(dcp == 2),
                                         perf_mode=DR)
                else:
                    for dc in range(DC):
                        nc.tensor.matmul(ps[:, :],
                                         mkvT[:, dc, lt * P:(lt + 1) * P],
                                         WvT[:, dc, :],
                                         start=(dc == 0), stop=(dc == DC - 1))
                nc.scalar.activation(v[:, lt, :], ps[:, :], COPY)
            return v

        def scores_kt(mkvT, tT, probsT, kt, fp8):
            ps = psA.tile([P, L], F32, tag="score", name="ps_sc")
            for qh in range(2):
                if fp8:
                    for dcp in (0, 2):
                        nc.tensor.matmul(
                            ps[:, qh * 512:(qh + 1) * 512],
                            mkvT[:, dcp:dcp + 2, kt * P:(kt + 1) * P],
                            tT[:, dcp:dcp + 2, qh * 512:(qh + 1) * 512],
                            start=(dcp == 0), stop=(dcp == 2), perf_mode=DR)
                else:
                    for dc in range(DC):
                        nc.tensor.matmul(
                            ps[:, qh * 512:(qh + 1) * 512],
                            mkvT[:, dc, kt * P:(kt + 1) * P],
                            tT[:, dc, qh * 512:(qh + 1) * 512],
                            start=(dc == 0), stop=(dc == DC - 1))
            nc.scalar.activation(probsT[:, kt, :], ps[:, :], EXP,
                                 scale=(SCALE / WSC if fp8 else SCALE),
                                 bias=ebias[:, :])

        def pv_qt(p, qt):
            """One PV qt-group of a pending unit p (normalized into pou)."""
            probsT, v, pou, recip, sums, kind, fp8 = p
            cross_b = kind[1] if (kind != "attn1" and kind[0] == "cross") else None
            po = psB.tile([P, A], F32, tag="mm", name="ps_pv")
            if fp8:
                for kcp in (0, 2, 4, 6):
                    w = probsT[:, kcp:kcp + 2, qt * P:(qt + 1) * P]
                    nc.tensor.matmul(po[:, :], w, v[:, kcp:kcp + 2, :],
                                     start=(kcp == 0), stop=(kcp == 6),
                                     perf_mode=DR)
                    nc.tensor.matmul(sums[:, qt:qt + 1], w, onesDR[:, :, :],
                                     start=(kcp == 0), stop=(kcp == 6),
                                     perf_mode=DR)
            else:
                for kc in range(KC):
                    w = probsT[:, kc, qt * P:(qt + 1) * P]
                    nc.tensor.matmul(po[:, :], w, v[:, kc, :],
                                     start=(kc == 0), stop=(kc == KC - 1))
                    nc.tensor.matmul(sums[:, qt:qt + 1], w, ones_bf[:, :],
                                     start=(kc == 0), stop=(kc == KC - 1))
            nc.vector.reciprocal(recip[:, qt:qt + 1], sums[:, qt:qt + 1])
            if cross_b is None:
                nc.vector.tensor_scalar_mul(pou[:, qt, :], po[:, :],
                                            recip[:, qt:qt + 1])
            else:
                out_c = smallp.tile([P, A], BF16, tag="out_c", name="out_c", bufs=8)
                nc.vector.tensor_scalar_mul(out_c[:, :], po[:, :],
                                            recip[:, qt:qt + 1])
                nc.sync.dma_start(
                    out=out[cross_b, qt * P:(qt + 1) * P, 3 * A:4 * A],
                    in_=out_c[:, :])

        # --- deferred combine: out = pou1 + pou2 + res (all bf16, DVE 2x) ----
        def combine_qt(pou1, pou2, blk, b, col, qt):
            res_t = smallp.tile([P, A], BF16, tag="res_t", name="res_t", bufs=6)
            nc.sync.dma_start(out=res_t[:, :],
                              in_=res[blk, b, qt * P:(qt + 1) * P, :])
            osum = smallp.tile([P, A], BF16, tag="osum", name="osum", bufs=6)
            nc.vector.tensor_add(osum[:, :], pou1[:, qt, :], pou2[:, qt, :])
            nc.vector.tensor_add(osum[:, :], osum[:, :], res_t[:, :])
            nc.sync.dma_start(
                out=out[b, qt * P:(qt + 1) * P, col * A:(col + 1) * A],
                in_=osum[:, :])
            if blk == 1:
                nc.sync.dma_start_transpose(
                    out=avT[b][:, :, qt * P:(qt + 1) * P],
                    in_=osum[:, :])

        def combine_pair(pou1, pou2, blk, b, col):
            for qt in range(LT):
                combine_qt(pou1, pou2, blk, b, col, qt)

        # --- unit schedule ----------------------------------------------------
        # order: blk0 (4 units), blk1 (4), blk2-b0 (2), cross-b0, cross-b1,
        # blk2-b1 (2)  -- crosses mid-stream, sym tail.
        blocks = [(0, "txt", "au", 0), (1, "vi", "au", 2), (2, "txt", "vi", 1)]

        def sym_units(blk, n1, n2, col, b):
            j0 = blk * 4
            return [
                dict(mq=(n2, b), mkv=(n1, b), wE=(j0 + 0, "E1"),
                     wV=(j0 + 2, "Wv1"), kind="attn1", fp8=True, prefetch=None),
                dict(mq=(n1, b), mkv=(n2, b), wE=(j0 + 1, "E2"),
                     wV=(j0 + 3, "Wv2"), kind=("attn2", blk, b, col),
                     fp8=True, prefetch=None),
            ]

        def cross_unit(b):
            return dict(mq=("avT", b), mkv=("txtb", b), wE=(12, "E1"),
                        wV=(13, "Wv1"), kind=("cross", b), fp8=False,
                        prefetch=None)

        units = []
        for b in range(BLOC):
            units += sym_units(0, "txt", "au", 0, b)
        for b in range(BLOC):
            units += sym_units(1, "vi", "au", 2, b)
        units += sym_units(2, "txt", "vi", 1, 0)
        units += [cross_unit(0), cross_unit(1)]
        units += sym_units(2, "txt", "vi", 1, 1)
        # weight prefetches: next block's weights at the previous block's start
        units[0]["prefetch"] = [(4, "E1"), (5, "E2"), (6, "Wv1"), (7, "Wv2")]
        units[4]["prefetch"] = [(8, "E1"), (9, "E2"), (10, "Wv1"), (11, "Wv2")]
        units[8]["prefetch"] = [(12, "E1"), (13, "Wv1")]

        # --- software pipeline -------------------------------------------
        # Unit step i: proj(i), pvA(i-1), scores(i) with pvB(i-1) interleaved,
        # then combines created at step <= i-1.
        pend_pv = None
        comb_queue = []       # [(pou1, pou2, kind, created_step)]
        prev_attn1 = None

        def finish_pv(p, step):
            nonlocal prev_attn1
            pou_p, kind_p = p[2], p[5]
            if kind_p == "attn1":
                prev_attn1 = pou_p
            elif kind_p[0] == "attn2":
                comb_queue.append((prev_attn1, pou_p, kind_p, step))

        for step, u in enumerate(units):
            if u["prefetch"]:
                for j, tag in u["prefetch"]:
                    use_w(j, tag)
            E = use_w(*u["wE"])
            Wv = use_w(*u["wV"])
            mq = avT[u["mq"][1]] if u["mq"][0] == "avT" else mtT[u["mq"]]
            mkv = txtb[u["mkv"][1]] if u["mkv"][0] == "txtb" else mtT[u["mkv"]]
            tT = proj_T(E, mq, u["fp8"])
            v = proj_N(Wv, mkv, u["fp8"])
            if pend_pv is not None:
                for qt in range(2):
                    pv_qt(pend_pv, qt)
            probsT = work.tile([P, KC, L], FP8 if u["fp8"] else BF16,
                               tag="probsT", name="probsT")
            # scores with pvB of the previous unit interleaved (hides exp)
            for kt in range(KC):
                scores_kt(mkv, tT, probsT, kt, u["fp8"])
                if pend_pv is not None and 1 <= kt <= 6:
                    pv_qt(pend_pv, kt + 1)
            if pend_pv is not None:
                finish_pv(pend_pv, step)
            while comb_queue and comb_queue[0][3] < step:
                pou1, pou2, kind, _ = comb_queue.pop(0)
                combine_pair(pou1, pou2, kind[1], kind[2], kind[3])
            pou = work.tile([P, LT, A], BF16, tag="pou", name="pou", bufs=3)
            recip = work.tile([P, LT], F32, tag="recip", name="recip", bufs=3)
            sums = psC.tile([P, LT], F32, tag="sums", name="sums")
            pend_pv = (probsT, v, pou, recip, sums, u["kind"], u["fp8"])
        # tail flush: interleave the last pair's combine with its PV per qt
        kind_last = pend_pv[5]
        pou_last = pend_pv[2]
        if kind_last != "attn1" and kind_last[0] == "attn2":
            _, blkL, bL, colL = kind_last
            for qt in range(LT):
                pv_qt(pend_pv, qt)
                combine_qt(prev_attn1, pou_last, blkL, bL, colL, qt)
        else:
            for qt in range(LT):
                pv_qt(pend_pv, qt)
            finish_pv(pend_pv, len(units))
        for pou1, pou2, kind, _ in comb_queue:
            combine_pair(pou1, pou2, kind[1], kind[2], kind[3])


_nc_cache = None
last_results = None


def _get_nc():
    global _nc_cache
    if _nc_cache is None:
        _nc_cache = _build()
    return _nc_cache


def kernel(**inputs):
    global last_results
    txt = np.asarray(inputs["txt"], dtype=np.float32)
    au = np.asarray(inputs["au"], dtype=np.float32)
    vi = np.asarray(inputs["vi"], dtype=np.float32)

    nat = {"txt": txt, "au": au, "vi": vi}
    mtn = {n: np.ascontiguousarray(v.transpose(0, 2, 1)) for n, v in nat.items()}
    mt8 = {n: v.astype(ml_dtypes.float8_e4m3) for n, v in mtn.items()}
    txt_bf = mtn["txt"].astype(ml_dtypes.bfloat16)

    g = {n: np.asarray(inputs[n], dtype=np.float32) for n in inputs}
    wlist = []
    for blk in ("ta", "va", "tv"):
        wlist += [
            WSC * (g[f"{blk}_qy"].T @ g[f"{blk}_kx"]),
            WSC * (g[f"{blk}_qx"].T @ g[f"{blk}_ky"]),
            WSC * g[f"{blk}_vx"].T,
            WSC * g[f"{blk}_vy"].T,
        ]
    wt_all = np.ascontiguousarray(np.stack(wlist)).astype(ml_dtypes.float8_e4m3)
    wtb_all = np.ascontiguousarray(np.stack(
        [g["tav_q"].T @ g["tav_k"], g["tav_v"].T])).astype(ml_dtypes.bfloat16)

    res_all = np.stack([txt + au, vi + au, txt + vi]).astype(ml_dtypes.bfloat16)

    in_maps = []
    for c in range(NCORES):
        sl = slice(c * BLOC, (c + 1) * BLOC)
        in_maps.append({
            "mt_txt": mt8["txt"][sl],
            "mt_au": mt8["au"][sl],
            "mt_vi": mt8["vi"][sl],
            "mt_txtb": txt_bf[sl],
            "res": np.ascontiguousarray(res_all[:, sl]),
            "wt": wt_all,
            "wtb": wtb_all,
        })

    nc = _get_nc()
    last_results = run_bass_kernel_spmd(nc, in_maps, core_ids=list(range(NCORES)))
    core_out = np.concatenate(
        [np.asarray(last_results.results[c]["out"]).astype(np.float32)
         for c in range(NCORES)], axis=0)
    return np.concatenate([txt, au, vi, core_out], axis=-1).astype(np.float32)


# revision 30
# speedup vs baseline: 1.0185x; 1.0185x over previous
"""v6: mixed-precision E-fused attention. Six symmetric attentions in fp8
(e4m3) DoubleRow; the sharp-softmax cross attention on bf16. Transposed-score
layout (exp emits probsT directly), softmax sums via interleaved N=1 matmuls,
PV normalized in the PSUM->SBUF copy, residual combines deferred two pipeline
stages with bf16 adds (DVE 2x), bf16 outputs upcast on host. PV of the
previous unit is interleaved into the scores loop to hide exp latency.
Data-parallel: 2 batches/core, 8 cores."""

import numpy as np
import ml_dtypes

from concourse import bacc, bass, tile, mybir
from concourse.bass_utils import run_bass_kernel_spmd

B, L, D = 16, 1024, 512
A = D
NCORES = 8
BLOC = B // NCORES
P = 128
DC = D // P          # 4 contraction chunks of 128
LT = L // P          # 8 row blocks
KC = L // P          # 8 key blocks
SCALE = float(1.0 / np.sqrt(np.float32(D)))
WSC = 16.0           # host weight pre-scale for fp8 (subnormal avoidance)
EB = -2.0            # exp bias: probs scaled by e^-2, cancels in softmax

F32 = mybir.dt.float32
BF16 = mybir.dt.bfloat16
FP8 = mybir.dt.float8e4
DR = mybir.MatmulPerfMode.DoubleRow
EXP = mybir.ActivationFunctionType.Exp
COPY = mybir.ActivationFunctionType.Copy


def _build():
    nc = bacc.Bacc("TRN2", target_bir_lowering=False, debug=False,
                   num_devices=NCORES)

    mt_txt = nc.dram_tensor("mt_txt", (BLOC, D, L), FP8, kind="ExternalInput").ap()
    mt_au = nc.dram_tensor("mt_au", (BLOC, D, L), FP8, kind="ExternalInput").ap()
    mt_vi = nc.dram_tensor("mt_vi", (BLOC, D, L), FP8, kind="ExternalInput").ap()
    mt_txtb = nc.dram_tensor("mt_txtb", (BLOC, D, L), BF16,
                             kind="ExternalInput").ap()
    res = nc.dram_tensor("res", (3, BLOC, L, D), BF16, kind="ExternalInput").ap()
    wt = nc.dram_tensor("wt", (12, D, A), FP8, kind="ExternalInput").ap()
    wtb = nc.dram_tensor("wtb", (2, D, A), BF16, kind="ExternalInput").ap()
    out = nc.dram_tensor("out", (BLOC, L, 4 * A), BF16, kind="ExternalOutput").ap()

    with tile.TileContext(nc) as tc:
        _body(nc, tc, mt_txt, mt_au, mt_vi, mt_txtb, res, wt, wtb, out)

    nc.compile()
    return nc


def _body(nc, tc, mt_txt, mt_au, mt_vi, mt_txtb, res, wt, wtb, out):
    mt_dram = {"txt": mt_txt, "au": mt_au, "vi": mt_vi}

    with (
        tc.tile_pool(name="persist", bufs=1) as persist,
        tc.tile_pool(name="wpool", bufs=2) as wpool,
        tc.tile_pool(name="work", bufs=2) as work,
        tc.tile_pool(name="small", bufs=3) as smallp,
        tc.tile_pool(name="ps_score", bufs=2, space=bass.MemorySpace.PSUM) as psA,
        tc.tile_pool(name="ps_mm", bufs=3, space=bass.MemorySpace.PSUM) as psB,
        tc.tile_pool(name="ps_sums", bufs=1, space=bass.MemorySpace.PSUM) as psC,
    ):
        # --- persistent tiles; first-needed inputs first, weights on the
        # scalar DMA queue so they issue in parallel with the sync queue ------
        mtT = {}

        def load_mt(name, b):
            t = persist.tile([P, DC, L], FP8, tag=f"mt_{name}{b}",
                             name=f"mt_{name}{b}")
            nc.sync.dma_start(
                out=t[:, :, :],
                in_=mt_dram[name][b].rearrange("(dc p) l -> p dc l", p=P))
            mtT[(name, b)] = t

        wtiles = {}

        def use_w(j, tag):
            # j >= 12 -> bf16 cross weights from wtb
            if j not in wtiles:
                if j >= 12:
                    t = wpool.tile([P, DC, A], BF16, tag=tag, name=f"w{j}")
                    nc.scalar.dma_start(
                        out=t[:, :, :],
                        in_=wtb[j - 12].rearrange("(dc p) a -> p dc a", p=P))
                else:
                    t = wpool.tile([P, DC, A], FP8, tag=tag, name=f"w{j}")
                    nc.scalar.dma_start(
                        out=t[:, :, :],
                        in_=wt[j].rearrange("(dc p) a -> p dc a", p=P))
                wtiles[j] = t
            return wtiles[j]

        def use_w_chunked(j, tag):
            t = wpool.tile([P, DC, A], FP8, tag=tag, name=f"w{j}")
            wsrc = wt[j].rearrange("(dc p) a -> p dc a", p=P)
            for h in (0, 2):
                nc.scalar.dma_start(out=t[:, h:h + 2, :], in_=wsrc[:, h:h + 2, :])
            wtiles[j] = t

        def load_mt_chunked(name, b):
            t = persist.tile([P, DC, L], FP8, tag=f"mt_{name}{b}",
                             name=f"mt_{name}{b}")
            msrc = mt_dram[name][b].rearrange("(dc p) l -> p dc l", p=P)
            for h in (0, 2):
                nc.sync.dma_start(out=t[:, h:h + 2, :], in_=msrc[:, h:h + 2, :])
            mtT[(name, b)] = t

        use_w_chunked(0, "E1")
        use_w_chunked(2, "Wv1")
        for j, tag in ((1, "E2"), (3, "Wv2")):
            use_w(j, tag)
        load_mt_chunked("txt", 0)
        load_mt_chunked("au", 0)
        load_mt("vi", 0)
        load_mt("txt", 1)
        load_mt("au", 1)
        load_mt("vi", 1)
        txtb = []
        for b in range(BLOC):
            t = persist.tile([P, DC, L], BF16, tag=f"txtb{b}", name=f"txtb{b}")
            nc.sync.dma_start(
                out=t[:, :, :],
                in_=mt_txtb[b].rearrange("(dc p) l -> p dc l", p=P))
            txtb.append(t)

        avT = [persist.tile([P, DC, L], BF16, tag=f"avT{b}", name=f"avT{b}")
               for b in range(BLOC)]
        onesDR = persist.tile([P, 2, 1], FP8, tag="onesDR", name="onesDR")
        nc.vector.memset(onesDR[:, :, :], WSC)
        ones_bf = persist.tile([P, 1], BF16, tag="ones_bf", name="ones_bf")
        nc.vector.memset(ones_bf[:, :], 1.0)
        ebias = persist.tile([P, 1], F32, tag="ebias", name="ebias")
        nc.vector.memset(ebias[:, :], EB)
        warm = psB.tile([P, 512], F32, tag="mm", name="warm")
        for _ in range(40):
            nc.tensor.matmul(warm[0:1, 0:1], ones_bf[:, :], ones_bf[:, :],
                             start=True, stop=True)

        # --- per-unit phases --------------------------------------------------
        def proj_T(E, mqT, fp8):
            tT = work.tile([P, DC, L], FP8 if fp8 else BF16, tag="tT", name="tT")
            for db in range(DC):
                for qh in range(2):
                    ps = psB.tile([P, 512], F32, tag="mm", name="ps_pt")
                    if fp8:
                        for dcp in (0, 2):
                            nc.tensor.matmul(
                                ps[:, :],
                                E[:, dcp:dcp + 2, db * P:(db + 1) * P],
                                mqT[:, dcp:dcp + 2, qh * 512:(qh + 1) * 512],
                                start=(dcp == 0), stop=(dcp == 2), perf_mode=DR)
                    else:
                        for dc in range(DC):
                            nc.tensor.matmul(
                                ps[:, :],
                                E[:, dc, db * P:(db + 1) * P],
                                mqT[:, dc, qh * 512:(qh + 1) * 512],
                                start=(dc == 0), stop=(dc == DC - 1))
                    nc.vector.tensor_copy(tT[:, db, qh * 512:(qh + 1) * 512],
                                          ps[:, :])
            return tT

        def proj_N(WvT, mkvT, fp8):
            v = work.tile([P, KC, A], FP8 if fp8 else BF16, tag="v", name="v")
            for lt in range(LT):
                ps = psB.tile([P, A], F32, tag="mm", name="ps_pn")
                if fp8:
                    for dcp in (0, 2):
                        nc.tensor.matmul(ps[:, :],
                                         mkvT[:, dcp:dcp + 2, lt * P:(lt + 1) * P],
                                         WvT[:, dcp:dcp + 2, :],
                                         start=(dcp == 0), stop=(dcp == 2),
                                         perf_mode=DR)
                else:
                    for dc in range(DC):
                        nc.tensor.matmul(ps[:, :],
                                         mkvT[:, dc, lt * P:(lt + 1) * P],
                                         WvT[:, dc, :],
                                         start=(dc == 0), stop=(dc == DC - 1))
                nc.scalar.activation(v[:, lt, :], ps[:, :], COPY)
            return v

        def scores_kt(mkvT, tT, probsT, kt, fp8):
            ps = psA.tile([P, L], F32, tag="score", name="ps_sc")
            for qh in range(2):
                if fp8:
                    for dcp in (0, 2):
                        nc.tensor.matmul(
                            ps[:, qh * 512:(qh + 1) * 512],
                            mkvT[:, dcp:dcp + 2, kt * P:(kt + 1) * P],
                            tT[:, dcp:dcp + 2, qh * 512:(qh + 1) * 512],
                            start=(dcp == 0), stop=(dcp == 2), perf_mode=DR)
                else:
                    for dc in range(DC):
                        nc.tensor.matmul(
                            ps[:, qh * 512:(qh + 1) * 512],
                            mkvT[:, dc, kt * P:(kt + 1) * P],
                            tT[:, dc, qh * 512:(qh + 1) * 512],
                            start=(dc == 0), stop=(dc == DC - 1))
            nc.scalar.activation(probsT[:, kt, :], ps[:, :], EXP,
                                 scale=(SCALE / WSC if fp8 else SCALE),
                                 bias=ebias[:, :])

        def pv_qt(p, qt):
            """One PV qt-group of a pending unit p (normalized into pou)."""
            probsT, v, pou, recip, sums, kind, fp8 = p
            cross_b = kind[1] if (kind != "attn1" and kind[0] == "cross") else None
            po = psB.tile([P, A], F32, tag="mm", name="ps_pv")
            if fp8:
                for kcp in (0, 2, 4, 6):
                    w = probsT[:, kcp:kcp + 2, qt * P:(qt + 1) * P]
                    nc.tensor.matmul(po[:, :], w, v[:, kcp:kcp + 2, :],
                                     start=(kcp == 0), stop=(kcp == 6),
                                     perf_mode=DR)
                    nc.tensor.matmul(sums[:, qt:qt + 1], w, onesDR[:, :, :],
                                     start=(kcp == 0), stop=(kcp == 6),
                                     perf_mode=DR)
            else:
                for kc in range(KC):
                    w = probsT[:, kc, qt * P:(qt + 1) * P]
                    nc.tensor.matmul(po[:, :], w, v[:, kc, :],
                                     start=(kc == 0), stop=(kc == KC - 1))
                    nc.tensor.matmul(sums[:, qt:qt + 1], w, ones_bf[:, :],
                                     start=(kc == 0), stop=(kc == KC - 1))
            nc.vector.reciprocal(recip[:, qt:qt + 1], sums[:, qt:qt + 1])
            if cross_b is None:
                nc.vector.tensor_scalar_mul(pou[:, qt, :], po[:, :],
                                            recip[:, qt:qt + 1])
            else:
                out_c = smallp.tile([P, A], BF16, tag="out_c", name="out_c", bufs=8)
                nc.vector.tensor_scalar_mul(out_c[:, :], po[:, :],
                                            recip[:, qt:qt + 1])
                nc.sync.dma_start(
                    out=out[cross_b, qt * P:(qt + 1) * P, 3 * A:4 * A],
                    in_=out_c[:, :])

        # --- deferred combine: out = pou1 + pou2 + res (all bf16, DVE 2x) ----
        def combine_qt(pou1, pou2, blk, b, col, qt):
            res_t = smallp.tile([P, A], BF16, tag="res_t", name="res_t", bufs=6)
            nc.scalar.dma_start(out=res_t[:, :],
                              in_=res[blk, b, qt * P:(qt + 1) * P, :])
            osum = smallp.tile([P, A], BF16, tag="osum", name="osum", bufs=6)
            nc.vector.tensor_add(osum[:, :], pou1[:, qt, :], pou2[:, qt, :])
            nc.vector.tensor_add(osum[:, :], osum[:, :], res_t[:, :])
            nc.sync.dma_start(
                out=out[b, qt * P:(qt + 1) * P, col * A:(col + 1) * A],
                in_=osum[:, :])
            if blk == 1:
                nc.sync.dma_start_transpose(
                    out=avT[b][:, :, qt * P:(qt + 1) * P],
                    in_=osum[:, :])

        def combine_pair(pou1, pou2, blk, b, col):
            for qt in range(LT):
                combine_qt(pou1, pou2, blk, b, col, qt)

        # --- unit schedule ----------------------------------------------------
        # order: blk0 (4 units), blk1 (4), blk2-b0 (2), cross-b0, cross-b1,
        # blk2-b1 (2)  -- crosses mid-stream, sym tail.
        blocks = [(0, "txt", "au", 0), (1, "vi", "au", 2), (2, "txt", "vi", 1)]

        def sym_units(blk, n1, n2, col, b):
            j0 = blk * 4
            return [
                dict(mq=(n2, b), mkv=(n1, b), wE=(j0 + 0, "E1"),
                     wV=(j0 + 2, "Wv1"), kind="attn1", fp8=True, prefetch=None),
                dict(mq=(n1, b), mkv=(n2, b), wE=(j0 + 1, "E2"),
                     wV=(j0 + 3, "Wv2"), kind=("attn2", blk, b, col),
                     fp8=True, prefetch=None),
            ]

        def cross_unit(b):
            return dict(mq=("avT", b), mkv=("txtb", b), wE=(12, "E1"),
                        wV=(13, "Wv1"), kind=("cross", b), fp8=False,
                        prefetch=None)

        units = []
        for b in range(BLOC):
            units += sym_units(0, "txt", "au", 0, b)
        for b in range(BLOC):
            units += sym_units(1, "vi", "au", 2, b)
        units += sym_units(2, "txt", "vi", 1, 0)
        units += [cross_unit(0), cross_unit(1)]
        units += sym_units(2, "txt", "vi", 1, 1)
        # weight prefetches: next block's weights at the previous block's start
        units[0]["prefetch"] = [(4, "E1"), (5, "E2"), (6, "Wv1"), (7, "Wv2")]
        units[4]["prefetch"] = [(8, "E1"), (9, "E2"), (10, "Wv1"), (11, "Wv2")]
        units[8]["prefetch"] = [(12, "E1"), (13, "Wv1")]

        # --- software pipeline -------------------------------------------
        # Unit step i: proj(i), pvA(i-1), scores(i) with pvB(i-1) interleaved,
        # then combines created at step <= i-1.
        pend_pv = None
        comb_queue = []       # [(pou1, pou2, kind, created_step)]
        prev_attn1 = None

        def finish_pv(p, step):
            nonlocal prev_attn1
            pou_p, kind_p = p[2], p[5]
            if kind_p == "attn1":
                prev_attn1 = pou_p
            elif kind_p[0] == "attn2":
                comb_queue.append((prev_attn1, pou_p, kind_p, step))

        for step, u in enumerate(units):
            if u["prefetch"]:
                for j, tag in u["prefetch"]:
                    use_w(j, tag)
            E = use_w(*u["wE"])
            Wv = use_w(*u["wV"])
            mq = avT[u["mq"][1]] if u["mq"][0] == "avT" else mtT[u["mq"]]
            mkv = txtb[u["mkv"][1]] if u["mkv"][0] == "txtb" else mtT[u["mkv"]]
            tT = proj_T(E, mq, u["fp8"])
            v = proj_N(Wv, mkv, u["fp8"])
            if pend_pv is not None:
                for qt in range(2):
                    pv_qt(pend_pv, qt)
            probsT = work.tile([P, KC, L], FP8 if u["fp8"] else BF16,
                               tag="probsT", name="probsT")
            # scores with pvB of the previous unit interleaved (hides exp)
            for kt in range(KC):
                scores_kt(mkv, tT, probsT, kt, u["fp8"])
                if pend_pv is not None and 1 <= kt <= 6:
                    pv_qt(pend_pv, kt + 1)
            if pend_pv is not None:
                finish_pv(pend_pv, step)
            while comb_queue and comb_queue[0][3] < step:
                pou1, pou2, kind, _ = comb_queue.pop(0)
                combine_pair(pou1, pou2, kind[1], kind[2], kind[3])
            pou = work.tile([P, LT, A], BF16, tag="pou", name="pou", bufs=3)
            recip = work.tile([P, LT], F32, tag="recip", name="recip", bufs=3)
            sums = psC.tile([P, LT], F32, tag="sums", name="sums")
            pend_pv = (probsT, v, pou, recip, sums, u["kind"], u["fp8"])
        # tail flush: interleave the last pair's combine with its PV per qt
        kind_last = pend_pv[5]
        pou_last = pend_pv[2]
        if kind_last != "attn1" and kind_last[0] == "attn2":
            _, blkL, bL, colL = kind_last
            for qt in range(LT):
                pv_qt(pend_pv, qt)
                combine_qt(prev_attn1, pou_last, blkL, bL, colL, qt)
        else:
            for qt in range(LT):
                pv_qt(pend_pv, qt)
            finish_pv(pend_pv, len(units))
        for pou1, pou2, kind, _ in comb_queue:
            combine_pair(pou1, pou2, kind[1], kind[2], kind[3])


_nc_cache = None
last_results = None


def _get_nc():
    global _nc_cache
    if _nc_cache is None:
        _nc_cache = _build()
    return _nc_cache


def kernel(**inputs):
    global last_results
    txt = np.asarray(inputs["txt"], dtype=np.float32)
    au = np.asarray(inputs["au"], dtype=np.float32)
    vi = np.asarray(inputs["vi"], dtype=np.float32)

    nat = {"txt": txt, "au": au, "vi": vi}
    mtn = {n: np.ascontiguousarray(v.transpose(0, 2, 1)) for n, v in nat.items()}
    mt8 = {n: v.astype(ml_dtypes.float8_e4m3) for n, v in mtn.items()}
    txt_bf = mtn["txt"].astype(ml_dtypes.bfloat16)

    g = {n: np.asarray(inputs[n], dtype=np.float32) for n in inputs}
    wlist = []
    for blk in ("ta", "va", "tv"):
        wlist += [
            WSC * (g[f"{blk}_qy"].T @ g[f"{blk}_kx"]),
            WSC * (g[f"{blk}_qx"].T @ g[f"{blk}_ky"]),
            WSC * g[f"{blk}_vx"].T,
            WSC * g[f"{blk}_vy"].T,
        ]
    wt_all = np.ascontiguousarray(np.stack(wlist)).astype(ml_dtypes.float8_e4m3)
    wtb_all = np.ascontiguousarray(np.stack(
        [g["tav_q"].T @ g["tav_k"], g["tav_v"].T])).astype(ml_dtypes.bfloat16)

    res_all = np.stack([txt + au, vi + au, txt + vi]).astype(ml_dtypes.bfloat16)

    in_maps = []
    for c in range(NCORES):
        sl = slice(c * BLOC, (c + 1) * BLOC)
        in_maps.append({
            "mt_txt": mt8["txt"][sl],
            "mt_au": mt8["au"][sl],
            "mt_vi": mt8["vi"][sl],
            "mt_txtb": txt_bf[sl],
            "res": np.ascontiguousarray(res_all[:, sl]),
            "wt": wt_all,
            "wtb": wtb_all,
        })

    nc = _get_nc()
    last_results = run_bass_kernel_spmd(nc, in_maps, core_ids=list(range(NCORES)))
    core_out = np.concatenate(
        [np.asarray(last_results.results[c]["out"]).astype(np.float32)
         for c in range(NCORES)], axis=0)
    return np.concatenate([txt, au, vi, core_out], axis=-1).astype(np.float32)
